# revision 4
# baseline (speedup 1.0000x reference)
"""Trainium2 Bass kernel for nn_Assigner (3D IoU anchor assignment), SPMD x8.

Strategy
--------
The 64 GT boxes are split across the 8 NeuronCores (8 GTs each); every core
scans all 262144 anchors against its local GTs. Per (anchor, gt) the kernel
mirrors the reference's f32 op order exactly (min/max/sub/mul sequence), so
threshold decisions match the jax reference bit-for-bit (validated: 0/262144
mismatches). Each core emits two per-anchor f32 sums:
  pos[a] = sum_j score05_j,  score05_j in {1.5,0.5,0}*5^-j (nonzero iff iou>=0.5)
  neg[a] = sum_j score03_j   (analogous for iou >= 0.3)
accumulated across the 8 local GTs by DMA-engine CCE ADD. The host combines
the 8 cores' sums, decodes the first-positive GT index from the leading
base-5 digit, and does the trivial 64-entry label/bbox table lookup.

Engine split (per core): DVE does the per-pair mins/products/threshold
deltas (11 ops/gt); ACT does relu + sign + score encoding; the otherwise
idle DMA engines accumulate; PE unused. F=2048 single-span, per-plane
staged input DMA, depth-2 software pipeline.

DVE per gt: 3x TS(max) + 3x STT(min-sub) + 2x TT(mult) + 3x STT(u,d05,d03)
ACT per gt: in-place relu [P,3F]; in-place sign [P,2F]; scaled-relu scores [P,2F]
DMA CCE: serialized ADD-accumulate chains for base-5 positional sums.
"""
import numpy as np

import concourse.bass as bass
import concourse.mybir as mybir
from concourse.bass_utils import run_bass_kernel_spmd

N = 262144
M = 64
P = 128
F = N // P            # 2048
GPC = M // 8
NPAR = 2

_cache = {}
_run_kwargs = {}
_last_res = None

PLANE_ORDER = [0, 3, 1, 4, 2, 5, 6]  # DMA issue order: z-pair, y-pair, x-pair, a1
PLANE_RANK = {k: i + 1 for i, k in enumerate(PLANE_ORDER)}  # 1-based completion count


def _build():
    nc = bass.Bass()
    f32 = mybir.dt.float32
    pk = nc.declare_dram_parameter("pk", [P, 7, F], f32, isOutput=False)
    gts = nc.declare_dram_parameter("gts", [10, GPC], f32, isOutput=False)
    pos = nc.declare_dram_parameter("pos", [P, F], f32, isOutput=True)
    neg = nc.declare_dram_parameter("neg", [P, F], f32, isOutput=True)

    Alu = mybir.AluOpType
    Act = mybir.ActivationFunctionType
    T = GPC

    from contextlib import ExitStack
    with ExitStack() as ctx:
        sb = lambda shape, name: ctx.enter_context(nc.sbuf_tensor(name, shape, f32))
        sem = lambda name: ctx.enter_context(nc.semaphore(name))
        grep = sb([P, 10 * GPC], 'grep')
        pkt = sb([P, 7, F], 'pkt')
        t_u = sb([P, F], 't_u')
        t_w = sb([P, NPAR, 3, F], 't_w')
        t_int = sb([P, F], 't_int')
        t_d = sb([P, NPAR, 2, F], 't_d')
        t_sc = sb([P, NPAR, 2, F], 't_sc')
        t_pacc = sb([P, F], 't_pacc')
        t_nacc = sb([P, F], 't_nacc')
        s_gt = sem("s_gt"); s_in = sem("s_in")
        s_dveA = sem("s_dveA"); s_dveB = sem("s_dveB")
        s_act = sem("s_act"); s_act2 = sem("s_act2")
        s_accP = sem("s_accP"); s_accN = sem("s_accN")
        s_out = sem("s_out")
        block = ctx.enter_context(nc.Block())

        def col(field, j):
            i = field * GPC + j
            return grep[:, i:i + 1]

        @block.gpsimd
        def _(gp):
            gp.dma_start(
                grep[:],
                gts[:].rearrange("a b -> (a b)")[None, :].partition_broadcast(P),
            ).then_inc(s_gt, 16)
            for t in range(T):
                par = t % NPAR
                if t == T - 1:
                    gp.wait_ge(s_dveB, T)
                else:
                    gp.wait_ge(s_act2, t + 1)
                if t == 0:
                    gp.dma_start(t_pacc[:], t_sc[:, par, 0]).then_inc(s_accP, 16)
                    gp.dma_start(t_nacc[:], t_sc[:, par, 1]).then_inc(s_accN, 16)
                else:
                    gp.wait_ge(s_accP, 16 * t)
                    gp.dma_start(t_pacc[:], t_sc[:, par, 0],
                                 accum_op=Alu.add).then_inc(s_accP, 16)
                    gp.wait_ge(s_accN, 16 * t)
                    gp.dma_start(t_nacc[:], t_sc[:, par, 1],
                                 accum_op=Alu.add).then_inc(s_accN, 16)

        @block.sync
        def _(sync):
            for k in PLANE_ORDER:
                sync.dma_start(pkt[:, k], pk[:, k]).then_inc(s_in, 16)
            sync.wait_ge(s_accP, 16 * T)
            sync.dma_start(pos[:], t_pacc[:]).then_inc(s_out, 16)
            sync.wait_ge(s_accN, 16 * T)
            sync.dma_start(neg[:], t_nacc[:]).then_inc(s_out, 16)
            sync.wait_ge(s_out, 32)

        @block.scalar
        def _(sc):
            def relu_step(t):
                par = t % NPAR
                sc.wait_ge(s_dveA, t + 1)
                if t >= NPAR:
                    sc.wait_ge(s_dveB, t - NPAR + 1)
                sc.activation(t_w[:, par], t_w[:, par], Act.Relu).then_inc(s_act, 1)

            def score_step(t):
                j = t
                par = t % NPAR
                sc.wait_ge(s_dveB, t + 1)
                if t >= NPAR:
                    sc.wait_ge(s_accP, 16 * (t - NPAR + 1))
                    sc.wait_ge(s_accN, 16 * (t - NPAR + 1))
                sc.activation(t_d[:, par], t_d[:, par], Act.Sign)
                sc.activation(t_sc[:, par], t_d[:, par], Act.Relu,
                              scale=col(7, j), bias=col(8, j)).then_inc(s_act2, 1)

            relu_step(0)
            for t in range(1, T):
                relu_step(t)
                score_step(t - 1)

        @block.vector
        def _(v):
            v.wait_ge(s_gt, 16)

            def stage_a(j):
                par = j % NPAR
                pl = lambda k: pkt[:, k]
                fine = j == 0
                if fine:
                    v.wait_ge(s_in, 16 * PLANE_RANK[0])
                v.tensor_scalar(t_u[:], pl(0), col(0, j), None, Alu.max)
                if fine:
                    v.wait_ge(s_in, 16 * PLANE_RANK[3])
                v.scalar_tensor_tensor(t_w[:, par, 0], pl(3), col(3, j), t_u[:], Alu.min, Alu.subtract)
                if fine:
                    v.wait_ge(s_in, 16 * PLANE_RANK[1])
                v.tensor_scalar(t_u[:], pl(1), col(1, j), None, Alu.max)
                if fine:
                    v.wait_ge(s_in, 16 * PLANE_RANK[4])
                v.scalar_tensor_tensor(t_w[:, par, 1], pl(4), col(4, j), t_u[:], Alu.min, Alu.subtract)
                if fine:
                    v.wait_ge(s_in, 16 * PLANE_RANK[2])
                v.tensor_scalar(t_u[:], pl(2), col(2, j), None, Alu.max)
                if fine:
                    v.wait_ge(s_in, 16 * PLANE_RANK[5])
                v.scalar_tensor_tensor(t_w[:, par, 2], pl(5), col(5, j), t_u[:], Alu.min, Alu.subtract) \
                    .then_inc(s_dveA, 1)

            def stage_b(j):
                t = j
                par = t % NPAR
                v.wait_ge(s_act, t + 1)
                if t == 0:
                    v.wait_ge(s_in, 16 * PLANE_RANK[6])
                if t >= NPAR:
                    v.wait_ge(s_act2, t - NPAR + 1)
                v.tensor_tensor(t_u[:], t_w[:, par, 0], t_w[:, par, 1], Alu.mult)
                v.tensor_tensor(t_int[:], t_u[:], t_w[:, par, 2], Alu.mult)
                v.scalar_tensor_tensor(t_u[:], pkt[:, 6], col(6, j), t_int[:], Alu.add, Alu.subtract)
                v.scalar_tensor_tensor(t_d[:, par, 0], t_u[:], 0.5, t_int[:], Alu.mult, Alu.subtract)
                ins = v.scalar_tensor_tensor(t_d[:, par, 1], t_u[:], 0.3, t_int[:], Alu.mult, Alu.subtract)
                if t == T - 1:
                    # drain fast: scores on DVE for the final gt (skip ACT round-trip)
                    v.wait_ge(s_accP, 16 * (T - 2))
                    v.wait_ge(s_accN, 16 * (T - 2))
                    v.tensor_scalar(t_sc[:, par, 0], t_d[:, par, 0], 0.0, col(9, j), Alu.is_le, Alu.mult)
                    ins = v.tensor_scalar(t_sc[:, par, 1], t_d[:, par, 1], 0.0, col(9, j), Alu.is_le, Alu.mult)
                ins.then_inc(s_dveB, 1)

            depth = NPAR - 1
            for t in range(depth):
                stage_a(t)
            for t in range(depth, T):
                stage_a(t)
                stage_b(t - depth)
            for t in range(T - depth, T):
                stage_b(t)
    return nc


def _host_pack(bboxes):
    b = np.ascontiguousarray(bboxes, dtype=np.float32)
    sz, sy, sx = b[:, 0], b[:, 1], b[:, 2]
    rz, ry, rx = b[:, 3], b[:, 4], b[:, 5]
    a1 = ((rz - sz) * (ry - sy)) * (rx - sx)
    planes = np.stack([sz, sy, sx, rz, ry, rx, a1], axis=0)  # [7, N]
    packed = planes.reshape(7, P, F).transpose(1, 0, 2)      # [P, 7, F]
    return np.ascontiguousarray(packed, dtype=np.float32)


W5 = (5.0 ** -np.arange(GPC)).astype(np.float32)


def _host_gts(gt_bboxes):
    g = np.ascontiguousarray(gt_bboxes, dtype=np.float32)
    A2 = ((g[:, 3] - g[:, 0]) * (g[:, 4] - g[:, 1])) * (g[:, 5] - g[:, 2])
    out = []
    for i in range(8):
        sl = slice(i * GPC, (i + 1) * GPC)
        full = np.stack([g[sl, 0], g[sl, 1], g[sl, 2], g[sl, 3], g[sl, 4], g[sl, 5],
                         A2[sl], -W5, 0.5 * W5, W5], axis=0)
        out.append(np.ascontiguousarray(full, dtype=np.float32))
    return out


def kernel(bboxes, gt_bboxes, gt_labels):
    bboxes = np.asarray(bboxes, dtype=np.float32)
    gt_bboxes = np.asarray(gt_bboxes, dtype=np.float32)
    gt_labels = np.asarray(gt_labels)

    if "nc" not in _cache:
        _cache["nc"] = _build()
    nc = _cache["nc"]

    packed = _host_pack(bboxes)
    gts_per_core = _host_gts(gt_bboxes)
    in_maps = [{"pk": packed, "gts": gts_per_core[i]} for i in range(8)]
    global _last_res
    res = run_bass_kernel_spmd(nc, in_maps, core_ids=list(range(8)), **_run_kwargs)
    _last_res = res

    psums = np.stack([res.results[i]["pos"].reshape(N) for i in range(8)]).astype(np.float64)
    nsums = np.stack([res.results[i]["neg"].reshape(N) for i in range(8)])

    has_pos = psums > 0
    pos_mask = has_pos.any(axis=0)
    first_core = np.argmax(has_pos, axis=0)
    s = np.take_along_axis(psums, first_core[None], axis=0)[0]
    s[~pos_mask] = 1.0
    j_local = np.round(-np.log(s) / np.log(5.0)).astype(np.int64)
    idx = first_core * GPC + j_local
    idx[~pos_mask] = 0
    neg_mask = ~np.any(nsums > 0, axis=0)

    zero = np.zeros((), dtype=gt_labels.dtype)
    minus1 = np.full((), -1, dtype=gt_labels.dtype)
    labels = np.where(pos_mask, gt_labels[idx], np.where(neg_mask, zero, minus1)).astype(gt_labels.dtype)
    bbox_out = np.where(pos_mask[:, None], gt_bboxes[idx], np.float32(-1.0)).astype(np.float32)
    return labels, bbox_out
